# revision 7
# baseline (speedup 1.0000x reference)
"""MoELoRA forward on 8 Trainium2 NeuronCores — fp8 split-precision version.

Data-parallel over tokens (512/core). All heavy matmuls in fp8e4 with
DoubleRow perf mode (2 K-chunks of 128 per instruction at 0.5 cyc/row).
Precision: W and x are each split hi+lo into two e4m3 tensors
(x ~= x8 + xl, 32*W^T ~= W8 + Wl); the base product uses three DR passes
x8@W8 + x8@Wl + xl@W8 (the xl@Wl term is ~0.13% and dropped), giving
~bf16-level accuracy at 0.75x bf16's PE cost. The global x32 weight scale
dodges e4m3's subnormal floor (W sigma ~ 1/32) and is divided out in the
PSUM->SBUF output copies. The LoRA path (~5% of output magnitude) runs
pure fp8: gates 32-scaled into exp(scale=1/32), t = x8@(32 U2^T),
tp = e4(t_psum * gate/16), out += tp @ e4(V2) (zero-padded DR).

Inputs stream in k-half chunks ordered by first consumption (x8/w8/xl
low halves first) so the PE starts real work ~4.3us in and never
starves. Bulk stores ride SWDGE kv_writeback (packed 16-rows-per-
descriptor format -> tiny DMA-engine occupancy, desc-gen overlapped on
the idle Pool engine); the final token block is split 256/256 with one
HWDGE store and one writeback so their completion chains overlap.
"""

import numpy as np
import ml_dtypes

_CACHE = {}

B, S, D_IN, D_OUT, E, R = 4, 1024, 1024, 1024, 8, 16
N_CORES = 8
N_TOK = B * S
TOK = N_TOK // N_CORES        # 512 tokens/core
ER = E * R                    # 128
HB = D_IN // 128              # 8 k-chunks
HP = HB // 2                  # 4 k-pairs
NB = TOK // 128               # 4 token blocks
SW = 32.0                     # global weight scale

BF16 = ml_dtypes.bfloat16
E4 = ml_dtypes.float8_e4m3


def _build():
    import concourse.tile as tile
    import concourse.bass_isa as bass_isa
    from concourse import bacc, mybir
    from contextlib import ExitStack

    f32 = mybir.dt.float32
    bf16 = mybir.dt.bfloat16
    i32 = mybir.dt.int32
    fp8 = mybir.dt.float8e4
    DR = mybir.MatmulPerfMode.DoubleRow
    Exp = mybir.ActivationFunctionType.Exp
    Copy = mybir.ActivationFunctionType.Copy

    nc = bacc.Bacc("TRN2", target_bir_lowering=False, debug=False,
                   num_devices=N_CORES)
    HKB = D_IN // 2  # 512, one k-half
    x8a_d = nc.dram_tensor("x8a", [HKB, TOK], fp8, kind="ExternalInput").ap()
    x8b_d = nc.dram_tensor("x8b", [HKB, TOK], fp8, kind="ExternalInput").ap()
    xla_d = nc.dram_tensor("xla", [HKB, TOK], fp8, kind="ExternalInput").ap()
    xlb_d = nc.dram_tensor("xlb", [HKB, TOK], fp8, kind="ExternalInput").ap()
    w8a0_d = nc.dram_tensor("w8a0", [HKB, 512], fp8, kind="ExternalInput").ap()
    w8a1_d = nc.dram_tensor("w8a1", [HKB, 512], fp8, kind="ExternalInput").ap()
    w8b_d = nc.dram_tensor("w8b", [D_IN, 512], fp8, kind="ExternalInput").ap()
    wla0_d = nc.dram_tensor("wla0", [HKB, 512], fp8, kind="ExternalInput").ap()
    wla1_d = nc.dram_tensor("wla1", [HKB, 512], fp8, kind="ExternalInput").ap()
    wlb_d = nc.dram_tensor("wlb", [D_IN, 512], fp8, kind="ExternalInput").ap()
    u28_d = nc.dram_tensor("u28", [128, HB * ER], fp8, kind="ExternalInput").ap()
    gw8_d = nc.dram_tensor("gw8", [128, HB * 2 * E], fp8, kind="ExternalInput").ap()
    v28_d = nc.dram_tensor("v28", [ER, D_OUT], fp8, kind="ExternalInput").ap()
    sel_d = nc.dram_tensor("sel", [E, 2 * ER], fp8, kind="ExternalInput").ap()
    idx_d = nc.dram_tensor("idx", [128, 8], i32, kind="ExternalInput").ap()
    # per-store output tensors (separate so Tile's conservative WAW
    # tracking can't chain one writeback onto the previous store's DMA)
    outA_d = nc.dram_tensor("outA", [TOK, 512], bf16, kind="ExternalOutput").ap()
    outB_d = [nc.dram_tensor(f"outB{i}", [128, 512], bf16, kind="ExternalOutput").ap()
              for i in range(NB - 1)]
    outF0_d = nc.dram_tensor("outF0", [128, 256], bf16, kind="ExternalOutput").ap()
    outF1_d = nc.dram_tensor("outF1", [128, 256], bf16, kind="ExternalOutput").ap()

    with tile.TileContext(nc) as tc, ExitStack() as ctx:
        sb = ctx.enter_context(tc.tile_pool(name="sb", bufs=1))
        ps = ctx.enter_context(tc.tile_pool(name="ps", bufs=8, space="PSUM"))
        osb = ctx.enter_context(tc.tile_pool(name="osb", bufs=1))

        x8 = sb.tile([128, HB, TOK], fp8, tag="x8")
        xl = sb.tile([128, HB, TOK], fp8, tag="xl")
        w8 = sb.tile([128, HB, D_OUT], fp8, tag="w8")
        wl = sb.tile([128, HB, D_OUT], fp8, tag="wl")
        u28 = sb.tile([128, HB, ER], fp8, tag="u28")
        gw8 = sb.tile([128, HB, 2 * E], fp8, tag="gw8")
        v28 = sb.tile([128, 2, D_OUT], fp8, tag="v28")
        selz = sb.tile([8, 2, ER], fp8, tag="selz")
        g8t = sb.tile([8, 2, TOK], fp8, tag="g8t")
        tpt = sb.tile([128, 2, TOK], fp8, tag="tpt")
        e_sb = sb.tile([8, TOK], f32, tag="e_sb")
        se_sb = sb.tile([8, TOK], f32, tag="se_sb")
        rec_sb = sb.tile([8, TOK], f32, tag="rec_sb")
        gp_sb = sb.tile([128, TOK], f32, tag="gp_sb")
        junk_sb = sb.tile([1, 512], bf16, tag="junk_sb")
        idx = sb.tile([128, 8], i32, tag="idx")
        # staging tiles for the stores
        oA = osb.tile([128, NB, 512], bf16, tag="oA")
        oB = [osb.tile([128, 1, 512], bf16, tag=f"oB{i}", name=f"oB{i}")
              for i in range(NB - 1)]
        oF0 = osb.tile([128, 256], bf16, tag="oF0")
        oF1 = osb.tile([128, 1, 256], bf16, tag="oF1")

        # memsets off the Pool engine (it runs the store desc-gens)
        nc.vector.memset(junk_sb[:], 0.0)
        nc.vector.memset(g8t[:], 0.0)
        nc.vector.memset(tpt[:], 0.0)
        nc.vector.memset(v28[:], 0.0)

        # ---- input DMAs (SP ring) in consumption order ----
        def ld(dst, src):
            nc.sync.dma_start(dst, src)

        ld(x8[:, 0:4, :], x8a_d.rearrange("(hb p) n -> p hb n", p=128))
        ld(w8[:, 0:4, 0:512], w8a0_d.rearrange("(hb p) o -> p hb o", p=128))
        ld(xl[:, 0:4, :], xla_d.rearrange("(hb p) n -> p hb n", p=128))
        ld(x8[:, 4:8, :], x8b_d.rearrange("(hb p) n -> p hb n", p=128))
        ld(gw8[:], gw8_d[:])
        ld(w8[:, 4:8, 0:512], w8a1_d.rearrange("(hb p) o -> p hb o", p=128))
        ld(xl[:, 4:8, :], xlb_d.rearrange("(hb p) n -> p hb n", p=128))
        ld(u28[:], u28_d[:])
        ld(selz[:], sel_d.rearrange("e (s r) -> e s r", s=2))
        ld(v28[:, 0, :], v28_d[:])
        ld(wl[:, 0:4, 0:512], wla0_d.rearrange("(hb p) o -> p hb o", p=128))
        ld(wl[:, 4:8, 0:512], wla1_d.rearrange("(hb p) o -> p hb o", p=128))
        ld(idx[:], idx_d[:])
        ld(w8[:, :, 512:1024], w8b_d.rearrange("(hb p) o -> p hb o", p=128))
        ld(wl[:, :, 512:1024], wlb_d.rearrange("(hb p) o -> p hb o", p=128))

        # ---- stores ride SWDGE kv_writeback (plain, Tile-managed sems):
        # the Pool engine generates descriptors (~1us, overlapped with PE)
        # and the packed 16-rows-per-descriptor format keeps the DMA-engine
        # occupancy tiny, so stores never contend with the input stream.
        def wb_store(dst, src_ap, nblk):
            nc.gpsimd.kv_writeback(
                dst.rearrange("(b p) (d n) -> b p d n", p=128, d=1),
                src_ap.rearrange("p (d b) n -> p d b n", d=1),
                idx[:, 0:nblk])

        # ---- PSUM banks ----
        misc = ps.tile([128, 512], f32, tag="ps", name="misc")
        t_ps = ps.tile([128, TOK], f32, tag="ps", name="t")
        accA = [ps.tile([128, 512], f32, tag="ps", name=f"accA{i}") for i in range(NB)]
        junk2 = ps.tile([1, 448], f32, tag="ps", name="junk2")

        # PE warmup during initial DMA dead-time
        N_WARM = 8
        nc.tensor.matmul(misc[0:1, 0:64], junk_sb[0:1, 0:1], junk_sb[:, 0:64],
                         start=True, stop=False, skip_group_check=True)
        for w in range(N_WARM):
            nc.tensor.matmul(misc[0:1, 0:448], junk_sb[0:1, 0:1], junk_sb[:, 0:448],
                             start=False, stop=(w == N_WARM - 1),
                             skip_group_check=True)

        def base_mm(acc, src_x, src_w, nb, oc, hp, start):
            nc.tensor.matmul(acc[nb][:],
                             src_x[:, 2 * hp:2 * hp + 2, nb * 128:(nb + 1) * 128],
                             src_w[:, 2 * hp:2 * hp + 2, oc * 512:(oc + 1) * 512],
                             start=start, stop=False,
                             perf_mode=DR, skip_group_check=True)

        def lora_mm(acc, nb, oc, start=False, stop=True):
            nc.tensor.matmul(acc[nb][:],
                             tpt[:, :, nb * 128:(nb + 1) * 128],
                             v28[:, :, oc * 512:(oc + 1) * 512],
                             start=start, stop=stop,
                             perf_mode=DR, skip_group_check=True)

        def dve_copy(dst, src):
            nc.vector.tensor_scalar(dst, src[:], 1.0 / SW, None,
                                    mybir.AluOpType.mult)

        def act_copy(dst, src):
            nc.scalar.activation(dst, src[:], Copy, scale=1.0 / SW)

        # ---- phase A (oc=0): pass order tracks DMA arrival:
        # w8a0, xla, [x8b full -> gate], w8a1, xlb, u28, wla halves ----
        oc = 0
        for hp in range(2):
            for nb in range(NB):
                base_mm(accA, x8, w8, nb, oc, hp, hp == 0)
        for hp in range(2):
            for nb in range(NB):
                base_mm(accA, xl, w8, nb, oc, hp, False)
        # gate logits gl[8, n] into misc rows 0:8 (DR pairs over k-chunks)
        for hp in range(HP):
            nc.tensor.matmul(misc[0:8, :], gw8[:, 2 * hp:2 * hp + 2, 0:E],
                             x8[:, 2 * hp:2 * hp + 2, :],
                             start=(hp == 0), stop=(hp == HP - 1),
                             perf_mode=DR, skip_group_check=True)
        # softmax chain: exp(gl/32) -> sum -> 1/sum -> g8 (fp8)
        nc.scalar.activation(e_sb[:], misc[0:8, :], Exp, scale=1.0 / SW)
        nc.gpsimd.partition_all_reduce(se_sb[:], e_sb[:], channels=8,
                                       reduce_op=bass_isa.ReduceOp.add)
        nc.vector.reciprocal(rec_sb[:], se_sb[:])
        with nc.allow_low_precision(reason="fp8 gate, lora path tolerance"):
            nc.vector.tensor_tensor(g8t[:, 0, :], e_sb[:], rec_sb[:],
                                    mybir.AluOpType.mult)
        for hp in range(2, HP):
            for nb in range(NB):
                base_mm(accA, x8, w8, nb, oc, hp, False)
        for hp in range(2, HP):
            for nb in range(NB):
                base_mm(accA, xl, w8, nb, oc, hp, False)
        for hp in range(HP):
            nc.tensor.matmul(t_ps[:], u28[:, 2 * hp:2 * hp + 2, :],
                             x8[:, 2 * hp:2 * hp + 2, :],
                             start=(hp == 0), stop=(hp == HP - 1),
                             perf_mode=DR, skip_group_check=True)
        # gate expand: gp[er, n] = g8[er//16, n]/16 (zero-padded DR), then
        # gp -> SBUF (ACT), tp = t * gp -> fp8 (DVE)
        gp_ps = ps.tile([128, TOK], f32, tag="ps", name="gp")
        nc.tensor.matmul(gp_ps[:], selz[:], g8t[:], start=True, stop=True,
                         perf_mode=DR, skip_group_check=True)
        nc.scalar.copy(gp_sb[:], gp_ps[:])
        with nc.allow_low_precision(reason="fp8 tp, lora path tolerance"):
            nc.vector.tensor_tensor(tpt[:, 0, :], t_ps[:], gp_sb[:],
                                    mybir.AluOpType.mult)
        for hp in range(HP):
            for nb in range(NB):
                base_mm(accA, x8, wl, nb, oc, hp, False)
        for nb in range(NB):
            lora_mm(accA, nb, oc)
        with nc.allow_low_precision(reason="bf16 output"):
            dve_copy(oA[:, 0, :], accA[0])
            act_copy(oA[:, 1, :], accA[1])
            dve_copy(oA[:, 2, :], accA[2])
            act_copy(oA[:, 3, :], accA[3])
        wb_store(outA_d, oA[:], NB)

        # ---- phase B (oc=1), bank-major; loraV leads each group so the
        # matmul right after the late w8b arrival isn't the PSUM opener ----
        oc = 1
        accB = [ps.tile([128, 512], f32, tag="ps", name=f"accB{i}")
                for i in range(NB - 1)]
        for nb in range(NB - 1):
            lora_mm(accB, nb, oc, start=True, stop=False)
            for hp in range(HP):
                base_mm(accB, x8, w8, nb, oc, hp, False)
            for hp in range(HP):
                base_mm(accB, xl, w8, nb, oc, hp, False)
            for hp in range(HP - 1):
                base_mm(accB, x8, wl, nb, oc, hp, False)
            hp = HP - 1
            nc.tensor.matmul(accB[nb][:],
                             x8[:, 2 * hp:2 * hp + 2, nb * 128:(nb + 1) * 128],
                             wl[:, 2 * hp:2 * hp + 2, oc * 512:(oc + 1) * 512],
                             start=False, stop=True,
                             perf_mode=DR, skip_group_check=True)
            with nc.allow_low_precision(reason="bf16 output"):
                if nb % 2 == 0:
                    dve_copy(oB[nb][:, 0, :], accB[nb])
                else:
                    act_copy(oB[nb][:, 0, :], accB[nb])
            wb_store(outB_d[nb], oB[nb][:], 1)

        # final token block split into two 256-col groups; group 0 goes out
        # via HWDGE (sync), group 1 via writeback, so the two completion
        # chains overlap instead of serializing on one dispatch path
        nb = NB - 1
        for i, (lo, hi) in enumerate(((0, 256), (256, 512))):
            fin = ps.tile([128, hi - lo], f32, tag="ps", name=f"fin{i}")
            lora_first = nc.tensor.matmul(
                fin[:], tpt[:, :, nb * 128:(nb + 1) * 128],
                v28[:, :, oc * 512 + lo:oc * 512 + hi],
                start=True, stop=False, perf_mode=DR, skip_group_check=True)
            for src_x, src_w in ((x8, w8), (xl, w8), (x8, wl)):
                for hp in range(HP):
                    last = (src_w is wl) and (hp == HP - 1)
                    nc.tensor.matmul(
                        fin[:],
                        src_x[:, 2 * hp:2 * hp + 2, nb * 128:(nb + 1) * 128],
                        src_w[:, 2 * hp:2 * hp + 2, oc * 512 + lo:oc * 512 + hi],
                        start=False, stop=last,
                        perf_mode=DR, skip_group_check=True)
            if i == 0:
                with nc.allow_low_precision(reason="bf16 output"):
                    act_copy(oF0[:], fin)
                nc.sync.dma_start(outF0_d[:], oF0[:])
            else:
                with nc.allow_low_precision(reason="bf16 output"):
                    dve_copy(oF1[:, 0, :], fin)
                wb_store(outF1_d, oF1[:], 1)

    nc.compile()
    return nc


def _get_nc():
    if "nc" not in _CACHE:
        _CACHE["nc"] = _build()
    return _CACHE["nc"]


def _q8(a):
    return np.ascontiguousarray(a).astype(E4)


def _prep_in_maps(x, weight, gate_w, lora_U, lora_V):
    xt = np.ascontiguousarray(x.reshape(N_TOK, D_IN).T)      # (D_IN, N_TOK) f32
    x8 = xt.astype(E4)
    xlr = xt - x8.astype(np.float32)
    xl8 = xlr.astype(E4)

    wTs = np.ascontiguousarray(weight.T) * SW                # (D_IN, D_OUT)
    w8 = wTs.astype(E4)
    wl8 = (wTs - w8.astype(np.float32)).astype(E4)

    u2T = np.ascontiguousarray(lora_U.reshape(ER, D_IN).T) * SW
    gwT = np.ascontiguousarray(gate_w.T) * SW
    # pre-arrange the small operands into the SBUF per-partition layout so the
    # DMA reads large contiguous elements (full bus rate instead of min-time)
    u2T = u2T.reshape(HB, 128, ER).transpose(1, 0, 2).reshape(128, HB * ER)
    gwp = np.concatenate([gwT, np.zeros_like(gwT)], axis=1)
    gwp = gwp.reshape(HB, 128, 2 * E).transpose(1, 0, 2).reshape(128, HB * 2 * E)
    v2 = np.ascontiguousarray(lora_V.transpose(0, 2, 1).reshape(ER, D_OUT))

    sel = np.zeros((E, 2, ER), dtype=np.float32)
    sel[:, 0, :] = np.repeat(np.eye(E, dtype=np.float32), R, axis=0).T / 16.0

    idx = np.zeros((128, 8), np.int32)   # writeback ctx offsets: all 0

    common = {
        "idx": np.ascontiguousarray(idx),
        "w8a0": _q8(w8[0:512, 0:512]), "w8a1": _q8(w8[512:1024, 0:512]),
        "w8b": _q8(w8[:, 512:1024]),
        "wla0": _q8(wl8[0:512, 0:512]), "wla1": _q8(wl8[512:1024, 0:512]),
        "wlb": _q8(wl8[:, 512:1024]),
        "u28": _q8(u2T),
        "gw8": _q8(gwp),
        "v28": _q8(v2),
        "sel": _q8(sel.reshape(E, 2 * ER)),
    }
    in_maps = []
    for c in range(N_CORES):
        m = dict(common)
        m["x8a"] = np.ascontiguousarray(x8[0:512, c * TOK:(c + 1) * TOK])
        m["x8b"] = np.ascontiguousarray(x8[512:1024, c * TOK:(c + 1) * TOK])
        m["xla"] = np.ascontiguousarray(xl8[0:512, c * TOK:(c + 1) * TOK])
        m["xlb"] = np.ascontiguousarray(xl8[512:1024, c * TOK:(c + 1) * TOK])
        in_maps.append(m)
    return in_maps


def kernel(x, weight, gate_w, lora_U, lora_V):
    from concourse import bass_utils

    x = np.asarray(x, dtype=np.float32)
    weight = np.asarray(weight, dtype=np.float32)
    gate_w = np.asarray(gate_w, dtype=np.float32)
    lora_U = np.asarray(lora_U, dtype=np.float32)
    lora_V = np.asarray(lora_V, dtype=np.float32)

    nc = _get_nc()
    in_maps = _prep_in_maps(x, weight, gate_w, lora_U, lora_V)
    res = bass_utils.run_bass_kernel_spmd(nc, in_maps, core_ids=list(range(N_CORES)))
    out = np.empty((N_TOK, D_OUT), dtype=np.float32)
    for c in range(N_CORES):
        r = res.results[c]
        o = out[c * TOK:(c + 1) * TOK]
        o[:, 0:512] = np.asarray(r["outA"], dtype=np.float32)
        for i in range(NB - 1):
            o[i * 128:(i + 1) * 128, 512:1024] = np.asarray(
                r[f"outB{i}"], dtype=np.float32)
        o[384:512, 512:768] = np.asarray(r["outF0"], dtype=np.float32)
        o[384:512, 768:1024] = np.asarray(r["outF1"], dtype=np.float32)
    return out.reshape(B, S, D_OUT)
